# revision 3
# baseline (speedup 1.0000x reference)
"""Single-head causal attention on 8 TRN2 NeuronCores, data-parallel over batch.

Problem: x [512, 256, 384] f32, Wq/Wk/Wv [384, 64] f32.
  q/k/v = x @ W;  S = q k^T / sqrt(384); causal softmax; out = P v.

Sharding: batch 512 -> 64 per core.  Host pre-transposes x so each device DMA
is contiguous; weights replicated.

Device: pair-granular software pipeline (pair = 2 batches), v2:
  iter i: [produce: QK(quad) on even / V(quad) on odd] PV(i-4) ST(i-2)
  - qkT = [Wq*scale | Wk] stationary over xT chunks; per-pair 1-bank PSUM
    tiles (bufs=2) so QK(h+1) only waits on the previous pair's cast.
  - ST is ROW-TILED: the two batches of a pair run CONCURRENTLY in PE row
    groups (0,0)/(64,0) (K=64 each).  Even batch: k moved to partitions 0:64;
    odd batch: q moved to partitions 64:128 (k stays in place).  Both moves
    are HWDGE SBUF->SBUF DMAs (off gpsimd).  st PSUM is [128, 2, 4, 128]
    (bank 0 = even batch, bank 1 = odd, 128-col pad so matmul outputs never
    cross a bank); one strided EXP covers both batches.
  - vT computed already transposed ([t, h]) with xT chunks stationary; the
    PSUM->SBUF copy runs on the SCALAR engine (activation Copy) to offload
    DVE; ones column seeded once per buffer slot (rowsums).
  - PV accumulates [t, 64+rowsum]; o cast (DVE) to bf16, DMA'd out
    UNNORMALIZED; softmax division on host in f32.
  - Head: quad 0's DMA split into 6 chunk-granular pieces in consumption
    order; ~14 warm-up matmuls on the wqk tile keep/raise HAM to K=8/8
    before real work lands.
  Rejected: fp8 anywhere (host sim: x=e4m3 3.1e-2, P=e4m3 2.2e-2,
  x=e3m4 1.56e-2 vs the 2e-2 budget; bf16 baseline 2.3e-3).
  PSUM banks: qk 2 + v 2 + st 2 + o 2 = 8.
"""

import numpy as np

import concourse.bacc as bacc
import concourse.bass as bass
import concourse.mybir as mybir
import concourse.tile as tile
from concourse.bass_utils import run_bass_kernel_spmd

N_CORES = 8
B, T, C, H = 512, 256, 384, 64
BPC = B // N_CORES          # 64 batches per core
PAIRS = BPC // 2            # 32 pair-iterations per core
QUADS = BPC // 4            # 16 quad-DMA groups
NCHUNK = C // 128           # 3 contraction chunks
SCALE = 1.0 / np.sqrt(C)    # note: reference scales by C**-0.5, not H**-0.5

F32 = mybir.dt.float32
BF16 = mybir.dt.bfloat16
EXP = mybir.ActivationFunctionType.Exp
COPY = mybir.ActivationFunctionType.Copy


def build_bass():
    nc = bacc.Bacc(None, target_bir_lowering=False, debug=False)
    x_in = nc.dram_tensor("xt", [QUADS, 128, NCHUNK, 4, T], BF16, kind="ExternalInput")
    wqk_in = nc.dram_tensor("wqk", [128, NCHUNK, 128], BF16, kind="ExternalInput")
    wv_in = nc.dram_tensor("wv", [128, NCHUNK, H], BF16, kind="ExternalInput")
    out_d = nc.dram_tensor("out", [QUADS, 128, 4, 2, H + 1], BF16, kind="ExternalOutput")

    with tile.TileContext(nc) as tc:
        with (
            tc.tile_pool(name="const", bufs=1) as const_pool,
            tc.tile_pool(name="xt", bufs=4) as xt_pool,
            tc.tile_pool(name="qk_sb", bufs=3) as qk_pool,
            tc.tile_pool(name="kq_sb", bufs=4) as kq_pool,
            tc.tile_pool(name="v_sb", bufs=4) as v_pool,
            tc.tile_pool(name="p_sb", bufs=4) as p_pool,
            tc.tile_pool(name="ob", bufs=3) as ob_pool,
            tc.tile_pool(name="qk_ps", bufs=2, space="PSUM") as qk_ps_pool,
            tc.tile_pool(name="v_ps", bufs=2, space="PSUM") as v_ps_pool,
            tc.tile_pool(name="st_ps", bufs=1, space="PSUM") as st_ps_pool,
            tc.tile_pool(name="o_ps", bufs=2, space="PSUM") as o_ps_pool,
        ):
            # weights first: wqk feeds both the warm-up matmuls and QK(0)
            wqk = const_pool.tile([128, NCHUNK, 128], BF16)
            nc.sync.dma_start(wqk[:], wqk_in[:])

            xts, qks, kqs, vs, pss, obs = {}, {}, {}, {}, {}, {}

            # initial prefetch: quad 0 in 6 chunk-granular pieces (QK h=0
            # consumes [:, n, 0:2, :] in n-order), quad 1 in halves
            for pf in range(min(4, QUADS)):
                xts[pf] = xt_pool.tile([128, NCHUNK, 4, T], BF16, tag="xt", name="xt")
            for n in range(NCHUNK):
                nc.sync.dma_start(xts[0][:, n, 0:2], x_in[0][:, n, 0:2])
            wv = const_pool.tile([128, NCHUNK, H], BF16)
            nc.sync.dma_start(wv[:], wv_in[:])
            for n in range(NCHUNK):
                nc.sync.dma_start(xts[0][:, n, 2:4], x_in[0][:, n, 2:4])
            if QUADS > 1:
                nc.sync.dma_start(xts[1][:, :, 0:2], x_in[1][:, :, 0:2])
                nc.sync.dma_start(xts[1][:, :, 2:4], x_in[1][:, :, 2:4])
            for pf in range(2, min(4, QUADS)):
                nc.sync.dma_start(xts[pf][:], x_in[pf])

            # HAM warm-up: ~14 dummy matmuls on the wqk tile (never read).
            # Rotates through the same PSUM slot the real QK tiles reuse.
            warm = qk_ps_pool.tile([128, 2, T], F32, tag="qk", name="qk_ps")
            for _ in range(14):
                nc.tensor.matmul(
                    warm[:, 0, :], wqk[:, 0, :], wqk[:, 0:2, :],
                    start=True, stop=True,
                )

            for i in range(PAIRS + 4):
                # ---- produce: QK(quad) on even iters, V(quad) on odd -------
                if i % 2 == 0 and i // 2 < QUADS:
                    q = i // 2
                    qks[q] = qk_pool.tile([128, 4, T], BF16, tag="qk", name="qk_sb")
                    for h in range(2):
                        # per-pair 1-bank PSUM tiles (bufs=2): QK(h+1) only
                        # waits for the CAST of h-1, not the whole quad
                        qk_ps = qk_ps_pool.tile([128, 2, T], F32, tag="qk", name="qk_ps")
                        for n in range(NCHUNK):
                            nc.tensor.matmul(
                                qk_ps[:],
                                wqk[:, n, :],
                                xts[q][:, n, 2 * h : 2 * h + 2, :],
                                start=(n == 0),
                                stop=(n == NCHUNK - 1),
                            )
                        nc.vector.tensor_copy(
                            qks[q][:, 2 * h : 2 * h + 2, :], qk_ps[:]
                        )
                        # ST row-tiling operand moves (HWDGE SBUF->SBUF):
                        # even batch's k down to partitions 0:64, odd batch's
                        # q up to partitions 64:128 (k_odd stays in qks).
                        m = 2 * q + h
                        kqs[m] = kq_pool.tile([128, T], BF16, tag="kq", name="kq_sb")
                        nc.sync.dma_start(kqs[m][0:64], qks[q][64:128, 2 * h])
                        nc.sync.dma_start(kqs[m][64:128], qks[q][0:64, 2 * h + 1])
                elif i % 2 == 1 and i // 2 < QUADS:
                    q = i // 2
                    # v, already transposed to [t, h]: xT chunk is stationary
                    v_ps = v_ps_pool.tile([128, 4, 2, H], F32, tag="v")
                    for jj in range(4):
                        for tb in range(2):
                            for n in range(NCHUNK):
                                nc.tensor.matmul(
                                    v_ps[:, jj, tb, :],
                                    xts[q][:, n, jj, bass.ts(tb, 128)],
                                    wv[:, n, :],
                                    start=(n == 0),
                                    stop=(n == NCHUNK - 1),
                                )
                    vs[q] = v_pool.tile([128, 4, 2, H + 1], BF16, tag="v", name="v_sb")
                    if q < 4:
                        # ones column for the PV rowsum: each pool slot keeps
                        # it forever (the copy below never touches col H)
                        nc.gpsimd.memset(vs[q][:, :, :, H : H + 1], 1.0)
                    # PSUM->SBUF drain on the scalar engine (DVE is busier)
                    nc.scalar.activation(vs[q][:, :, :, 0:H], v_ps[:], COPY)
                    del xts[q]
                    if q + 4 < QUADS:
                        xts[q + 4] = xt_pool.tile(
                            [128, NCHUNK, 4, T], BF16, tag="xt", name="xt"
                        )
                        nc.sync.dma_start(xts[q + 4][:], x_in[q + 4])

                # ---- PV(i-4) + writeback (mask select is 2 iterations old) -
                w = i - 4
                if 0 <= w < PAIRS:
                    qw, hw = divmod(w, 2)
                    o_ps = o_ps_pool.tile([128, 2, 2, H + 1], F32, tag="o")
                    for jl in range(2):
                        p = pss[w]
                        v = vs[qw]
                        jj = 2 * hw + jl
                        nc.tensor.matmul(
                            o_ps[:, jl, 0, :], p[:, jl, 0, :], v[:, jj, 0, :],
                            start=True, stop=True,
                        )
                        nc.tensor.matmul(
                            o_ps[:, jl, 1, :], p[:, jl, 2, :], v[:, jj, 0, :],
                            start=True, stop=False,
                        )
                        nc.tensor.matmul(
                            o_ps[:, jl, 1, :], p[:, jl, 1, :], v[:, jj, 1, :],
                            start=False, stop=True,
                        )
                    if hw == 0:
                        obs[qw] = ob_pool.tile(
                            [128, 4, 2, H + 1], BF16, tag="ob", name="ob"
                        )
                        nc.vector.tensor_copy(obs[qw][:, 0:2], o_ps[:])
                    else:
                        nc.vector.tensor_copy(obs[qw][:, 2:4], o_ps[:])
                        nc.sync.dma_start(out_d[qw], obs[qw][:])
                        del obs[qw], vs[qw]
                    del pss[w]

                # ---- ST(i-2) row-tiled + exp + causal mask -----------------
                m = i - 2
                if 0 <= m < PAIRS:
                    qm, hm = divmod(m, 2)
                    # bank 0 = even batch (rows 0:64), bank 1 = odd (64:128);
                    # 4th 128-col slot is padding so MMs never cross a bank
                    st = st_ps_pool.tile([128, 2, 4, 128], F32, tag="st")
                    q_e = qks[qm][0:64, 2 * hm]          # [64, 256] @ base 0
                    k_e = kqs[m][0:64]                   # [64, 256] @ base 0
                    q_o = kqs[m][64:128]                 # [64, 256] @ base 64
                    k_o = qks[qm][64:128, 2 * hm + 1]    # [64, 256] @ base 64
                    # interleave even/odd so the two row groups overlap;
                    # blocks: 0=(s0,t0) tri, 2=(s0,t1) full, 1=(s1,t1) tri
                    for out_blk, ks_, qs_ in (
                        (0, slice(0, 128), slice(0, 128)),
                        (2, slice(0, 128), slice(128, T)),
                        (1, slice(128, T), slice(128, T)),
                    ):
                        nc.tensor.matmul(
                            st[:, 0, out_blk, :], k_e[:, ks_], q_e[:, qs_],
                            start=True, stop=True,
                        )
                        nc.tensor.matmul(
                            st[:, 1, out_blk, :], k_o[:, ks_], q_o[:, qs_],
                            start=True, stop=True,
                        )
                    pss[m] = p_pool.tile([128, 2, 3, 128], BF16, tag="p", name="p_sb")
                    nc.scalar.activation(pss[m][:], st[:, :, 0:3, :], EXP)
                    # zero s > t in both triangular blocks of both batches:
                    # keep where col - partition >= 0
                    nc.gpsimd.affine_select(
                        out=pss[m][:, :, 0:2, :],
                        in_=pss[m][:, :, 0:2, :],
                        compare_op=mybir.AluOpType.is_ge,
                        fill=0.0,
                        base=0,
                        pattern=[[0, 2], [0, 2], [1, 128]],
                        channel_multiplier=-1,
                    )
                    del kqs[m]
                    if hm == 1:
                        del qks[qm]

    nc.finalize()
    return nc


_CACHED = {}


def _get_nc():
    if "nc" not in _CACHED:
        _CACHED["nc"] = build_bass()
    return _CACHED["nc"]


def prep_inputs(x, Wq, Wk, Wv):
    import ml_dtypes

    bf16 = ml_dtypes.bfloat16
    x = np.ascontiguousarray(x, dtype=np.float32)
    wqk = np.concatenate([Wq * SCALE, Wk], axis=1).astype(np.float32)  # [384, 128]
    wqk_t = np.ascontiguousarray(
        wqk.reshape(NCHUNK, 128, 128).transpose(1, 0, 2).astype(bf16)
    )
    wv_t = np.ascontiguousarray(
        Wv.astype(np.float32).reshape(NCHUNK, 128, H).transpose(1, 0, 2).astype(bf16)
    )

    in_maps = []
    for c in range(N_CORES):
        xs = x[c * BPC : (c + 1) * BPC]  # [64, 256, 384]
        # [q, jj, t, n, p] -> [q, p, n, jj, t]  (partition-major for the DMA)
        xt = np.ascontiguousarray(
            xs.reshape(QUADS, 4, T, NCHUNK, 128).transpose(0, 4, 3, 1, 2).astype(bf16)
        )
        in_maps.append({"xt": xt, "wqk": wqk_t, "wv": wv_t})
    return in_maps


def postprocess(results):
    outs = []
    for c in range(N_CORES):
        od = results[c]["out"].astype(np.float32)  # [QUADS, 128p, 4jj, 2n, H+1]
        o = od[..., 0:H] / od[..., H : H + 1]
        outs.append(o.transpose(0, 2, 3, 1, 4).reshape(BPC, T, H))
    return np.concatenate(outs, axis=0).astype(np.float32)


def kernel(x, Wq, Wk, Wv):
    in_maps = prep_inputs(x, Wq, Wk, Wv)
    res = run_bass_kernel_spmd(_get_nc(), in_maps, core_ids=list(range(N_CORES)))
    return postprocess(res.results)


# revision 4
# speedup vs baseline: 1.2611x; 1.2611x over previous
"""Single-head causal attention on 8 TRN2 NeuronCores, data-parallel over batch.

Problem: x [512, 256, 384] f32, Wq/Wk/Wv [384, 64] f32.
  q/k/v = x @ W;  S = q k^T / sqrt(384); causal softmax; out = P v.

Sharding: batch 512 -> 64 per core.  Host pre-transposes x so each device DMA
is contiguous; weights replicated.

Device: pair-granular software pipeline (pair = 2 batches), v2:
  iter i: [produce: QK(quad) on even / V(quad) on odd] PV(i-5) ST(i-3)
  - qkT = [Wq*scale | Wk] stationary over xT chunks; per-pair 1-bank PSUM
    tiles (bufs=2) so QK(h+1) only waits on the previous pair's cast.
  - ST is ROW-TILED: the two batches of a pair run CONCURRENTLY in PE row
    groups (0,0)/(64,0) (K=64 each).  Even batch: k moved to partitions 0:64;
    odd batch: q moved to partitions 64:128 (k stays in place).  Both moves
    are HWDGE SBUF->SBUF DMAs (off gpsimd).  st PSUM is [128, 2, 4, 128]
    (bank 0 = even batch, bank 1 = odd, 128-col pad so matmul outputs never
    cross a bank); one strided EXP covers both batches.
  - vT computed already transposed ([t, h]) with xT chunks stationary; the
    PSUM->SBUF copy runs on the SCALAR engine (activation Copy) to offload
    DVE; ones column seeded once per buffer slot (rowsums).
  - PV accumulates [t, 64+rowsum]; o cast (DVE) to bf16, DMA'd out
    UNNORMALIZED; softmax division on host in f32.
  - Head: quad 0's DMA split into 6 chunk-granular pieces in consumption
    order; ~14 warm-up matmuls on the wqk tile keep/raise HAM to K=8/8
    before real work lands.
  Rejected: fp8 anywhere (host sim: x=e4m3 3.1e-2, P=e4m3 2.2e-2,
  x=e3m4 1.56e-2 vs the 2e-2 budget; bf16 baseline 2.3e-3).
  PSUM banks: qk 2 + v 2 + st 2 + o 2 = 8.
"""

import numpy as np

import concourse.bacc as bacc
import concourse.bass as bass
import concourse.mybir as mybir
import concourse.tile as tile
from concourse.bass_utils import run_bass_kernel_spmd

N_CORES = 8
B, T, C, H = 512, 256, 384, 64
BPC = B // N_CORES          # 64 batches per core
PAIRS = BPC // 2            # 32 pair-iterations per core
QUADS = BPC // 4            # 16 quad-DMA groups
NCHUNK = C // 128           # 3 contraction chunks
SCALE = 1.0 / np.sqrt(C)    # note: reference scales by C**-0.5, not H**-0.5

F32 = mybir.dt.float32
BF16 = mybir.dt.bfloat16
EXP = mybir.ActivationFunctionType.Exp
COPY = mybir.ActivationFunctionType.Copy


def build_bass():
    nc = bacc.Bacc(None, target_bir_lowering=False, debug=False)
    x_in = nc.dram_tensor("xt", [QUADS, 128, NCHUNK, 4, T], BF16, kind="ExternalInput")
    wqk_in = nc.dram_tensor("wqk", [128, NCHUNK, 128], BF16, kind="ExternalInput")
    wv_in = nc.dram_tensor("wv", [128, NCHUNK, H], BF16, kind="ExternalInput")
    out_d = nc.dram_tensor("out", [QUADS, 128, 4, 2, H + 1], BF16, kind="ExternalOutput")

    with tile.TileContext(nc) as tc:
        with (
            tc.tile_pool(name="const", bufs=1) as const_pool,
            tc.tile_pool(name="xt", bufs=4) as xt_pool,
            tc.tile_pool(name="qk_sb", bufs=4) as qk_pool,
            tc.tile_pool(name="kq_sb", bufs=4) as kq_pool,
            tc.tile_pool(name="v_sb", bufs=4) as v_pool,
            tc.tile_pool(name="p_sb", bufs=4) as p_pool,
            tc.tile_pool(name="ob", bufs=3) as ob_pool,
            tc.tile_pool(name="qk_ps", bufs=2, space="PSUM") as qk_ps_pool,
            tc.tile_pool(name="v_ps", bufs=2, space="PSUM") as v_ps_pool,
            tc.tile_pool(name="st_ps", bufs=1, space="PSUM") as st_ps_pool,
            tc.tile_pool(name="o_ps", bufs=2, space="PSUM") as o_ps_pool,
        ):
            # weights first: wqk feeds both the warm-up matmuls and QK(0)
            wqk = const_pool.tile([128, NCHUNK, 128], BF16)
            nc.sync.dma_start(wqk[:], wqk_in[:])

            xts, qks, kqs, vs, pss, obs = {}, {}, {}, {}, {}, {}

            # initial prefetch: quad 0 in 6 chunk-granular pieces (QK h=0
            # consumes [:, n, 0:2, :] in n-order), quad 1 in halves
            for pf in range(min(4, QUADS)):
                xts[pf] = xt_pool.tile([128, NCHUNK, 4, T], BF16, tag="xt", name="xt")
            for n in range(NCHUNK):
                nc.sync.dma_start(xts[0][:, n, 0:2], x_in[0][:, n, 0:2])
            wv = const_pool.tile([128, NCHUNK, H], BF16)
            nc.sync.dma_start(wv[:], wv_in[:])
            for n in range(NCHUNK):
                nc.sync.dma_start(xts[0][:, n, 2:4], x_in[0][:, n, 2:4])
            if QUADS > 1:
                nc.sync.dma_start(xts[1][:, :, 0:2], x_in[1][:, :, 0:2])
                nc.sync.dma_start(xts[1][:, :, 2:4], x_in[1][:, :, 2:4])
            for pf in range(2, min(4, QUADS)):
                nc.sync.dma_start(xts[pf][:], x_in[pf])

            # HAM warm-up: ~14 dummy matmuls on the wqk tile (never read).
            # Rotates through the same PSUM slot the real QK tiles reuse.
            warm = qk_ps_pool.tile([128, 2, T], F32, tag="qk", name="qk_ps")
            for _ in range(14):
                nc.tensor.matmul(
                    warm[:, 0, :], wqk[:, 0, :], wqk[:, 0:2, :],
                    start=True, stop=True,
                )

            for i in range(PAIRS + 5):
                # ---- produce: QK(quad) on even iters, V(quad) on odd -------
                if i % 2 == 0 and i // 2 < QUADS:
                    q = i // 2
                    qks[q] = qk_pool.tile([128, 4, T], BF16, tag="qk", name="qk_sb")
                    for h in range(2):
                        # per-pair 1-bank PSUM tiles (bufs=2): QK(h+1) only
                        # waits for the CAST of h-1, not the whole quad
                        qk_ps = qk_ps_pool.tile([128, 2, T], F32, tag="qk", name="qk_ps")
                        for n in range(NCHUNK):
                            nc.tensor.matmul(
                                qk_ps[:],
                                wqk[:, n, :],
                                xts[q][:, n, 2 * h : 2 * h + 2, :],
                                start=(n == 0),
                                stop=(n == NCHUNK - 1),
                            )
                        nc.vector.tensor_copy(
                            qks[q][:, 2 * h : 2 * h + 2, :], qk_ps[:]
                        )
                        # ST row-tiling operand moves (HWDGE SBUF->SBUF):
                        # even batch's k down to partitions 0:64, odd batch's
                        # q up to partitions 64:128 (k_odd stays in qks).
                        m = 2 * q + h
                        kqs[m] = kq_pool.tile([128, T], BF16, tag="kq", name="kq_sb")
                        nc.sync.dma_start(kqs[m][0:64], qks[q][64:128, 2 * h])
                        nc.scalar.dma_start(kqs[m][64:128], qks[q][0:64, 2 * h + 1])
                elif i % 2 == 1 and i // 2 < QUADS:
                    q = i // 2
                    # v, already transposed to [t, h]: xT chunk is stationary
                    v_ps = v_ps_pool.tile([128, 4, 2, H], F32, tag="v")
                    for jj in range(4):
                        for tb in range(2):
                            for n in range(NCHUNK):
                                nc.tensor.matmul(
                                    v_ps[:, jj, tb, :],
                                    xts[q][:, n, jj, bass.ts(tb, 128)],
                                    wv[:, n, :],
                                    start=(n == 0),
                                    stop=(n == NCHUNK - 1),
                                )
                    vs[q] = v_pool.tile([128, 4, 2, H + 1], BF16, tag="v", name="v_sb")
                    if q < 4:
                        # ones column for the PV rowsum: each pool slot keeps
                        # it forever (the copy below never touches col H)
                        nc.gpsimd.memset(vs[q][:, :, :, H : H + 1], 1.0)
                    # PSUM->SBUF drain on the scalar engine (DVE is busier)
                    nc.scalar.activation(vs[q][:, :, :, 0:H], v_ps[:], COPY)
                    del xts[q]
                    if q + 4 < QUADS:
                        xts[q + 4] = xt_pool.tile(
                            [128, NCHUNK, 4, T], BF16, tag="xt", name="xt"
                        )
                        nc.sync.dma_start(xts[q + 4][:], x_in[q + 4])

                # ---- PV(i-4) + writeback (mask select is 2 iterations old) -
                w = i - 5
                if 0 <= w < PAIRS:
                    qw, hw = divmod(w, 2)
                    o_ps = o_ps_pool.tile([128, 2, 2, H + 1], F32, tag="o")
                    for jl in range(2):
                        p = pss[w]
                        v = vs[qw]
                        jj = 2 * hw + jl
                        nc.tensor.matmul(
                            o_ps[:, jl, 0, :], p[:, jl, 0, :], v[:, jj, 0, :],
                            start=True, stop=True,
                        )
                        nc.tensor.matmul(
                            o_ps[:, jl, 1, :], p[:, jl, 2, :], v[:, jj, 0, :],
                            start=True, stop=False,
                        )
                        nc.tensor.matmul(
                            o_ps[:, jl, 1, :], p[:, jl, 1, :], v[:, jj, 1, :],
                            start=False, stop=True,
                        )
                    if hw == 0:
                        obs[qw] = ob_pool.tile(
                            [128, 4, 2, H + 1], BF16, tag="ob", name="ob"
                        )
                        nc.vector.tensor_copy(obs[qw][:, 0:2], o_ps[:])
                    else:
                        nc.vector.tensor_copy(obs[qw][:, 2:4], o_ps[:])
                        nc.sync.dma_start(out_d[qw], obs[qw][:])
                        del obs[qw], vs[qw]
                    del pss[w]

                # ---- ST(i-2) row-tiled + exp + causal mask -----------------
                m = i - 3
                if 0 <= m < PAIRS:
                    qm, hm = divmod(m, 2)
                    # bank 0 = even batch (rows 0:64), bank 1 = odd (64:128);
                    # 4th 128-col slot is padding so MMs never cross a bank
                    st = st_ps_pool.tile([128, 2, 4, 128], F32, tag="st")
                    q_e = qks[qm][0:64, 2 * hm]          # [64, 256] @ base 0
                    k_e = kqs[m][0:64]                   # [64, 256] @ base 0
                    q_o = kqs[m][64:128]                 # [64, 256] @ base 64
                    k_o = qks[qm][64:128, 2 * hm + 1]    # [64, 256] @ base 64
                    # interleave even/odd so the two row groups overlap;
                    # blocks: 0=(s0,t0) tri, 2=(s0,t1) full, 1=(s1,t1) tri
                    for out_blk, ks_, qs_ in (
                        (0, slice(0, 128), slice(0, 128)),
                        (2, slice(0, 128), slice(128, T)),
                        (1, slice(128, T), slice(128, T)),
                    ):
                        nc.tensor.matmul(
                            st[:, 0, out_blk, :], k_e[:, ks_], q_e[:, qs_],
                            start=True, stop=True,
                        )
                        nc.tensor.matmul(
                            st[:, 1, out_blk, :], k_o[:, ks_], q_o[:, qs_],
                            start=True, stop=True,
                        )
                    pss[m] = p_pool.tile([128, 2, 3, 128], BF16, tag="p", name="p_sb")
                    nc.scalar.activation(pss[m][:], st[:, :, 0:3, :], EXP)
                    # zero s > t in both triangular blocks of both batches:
                    # keep where col - partition >= 0
                    nc.gpsimd.affine_select(
                        out=pss[m][:, :, 0:2, :],
                        in_=pss[m][:, :, 0:2, :],
                        compare_op=mybir.AluOpType.is_ge,
                        fill=0.0,
                        base=0,
                        pattern=[[0, 2], [0, 2], [1, 128]],
                        channel_multiplier=-1,
                    )
                    del kqs[m]
                    if hm == 1:
                        del qks[qm]

    nc.finalize()
    return nc


_CACHED = {}


def _get_nc():
    if "nc" not in _CACHED:
        _CACHED["nc"] = build_bass()
    return _CACHED["nc"]


def prep_inputs(x, Wq, Wk, Wv):
    import ml_dtypes

    bf16 = ml_dtypes.bfloat16
    x = np.ascontiguousarray(x, dtype=np.float32)
    wqk = np.concatenate([Wq * SCALE, Wk], axis=1).astype(np.float32)  # [384, 128]
    wqk_t = np.ascontiguousarray(
        wqk.reshape(NCHUNK, 128, 128).transpose(1, 0, 2).astype(bf16)
    )
    wv_t = np.ascontiguousarray(
        Wv.astype(np.float32).reshape(NCHUNK, 128, H).transpose(1, 0, 2).astype(bf16)
    )

    in_maps = []
    for c in range(N_CORES):
        xs = x[c * BPC : (c + 1) * BPC]  # [64, 256, 384]
        # [q, jj, t, n, p] -> [q, p, n, jj, t]  (partition-major for the DMA)
        xt = np.ascontiguousarray(
            xs.reshape(QUADS, 4, T, NCHUNK, 128).transpose(0, 4, 3, 1, 2).astype(bf16)
        )
        in_maps.append({"xt": xt, "wqk": wqk_t, "wv": wv_t})
    return in_maps


def postprocess(results):
    outs = []
    for c in range(N_CORES):
        od = results[c]["out"].astype(np.float32)  # [QUADS, 128p, 4jj, 2n, H+1]
        o = od[..., 0:H] / od[..., H : H + 1]
        outs.append(o.transpose(0, 2, 3, 1, 4).reshape(BPC, T, H))
    return np.concatenate(outs, axis=0).astype(np.float32)


def kernel(x, Wq, Wk, Wv):
    in_maps = prep_inputs(x, Wq, Wk, Wv)
    res = run_bass_kernel_spmd(_get_nc(), in_maps, core_ids=list(range(N_CORES)))
    return postprocess(res.results)
